# revision 37
# baseline (speedup 1.0000x reference)
"""Local (windowed) attention scores kernel for Trainium2, 8 NeuronCores — v2.

Computes softmax(Q_win @ [K_prev|K_self|K_next]^T / sqrt(d)) per 128-wide
window, drops windows 2 and 34, zeros padded edge regions of windows 0/63.
Data-parallel over collapsed batch*heads (32 -> 4 per core).

v2 design (vs baseline v1 at ~430us):
 - All device I/O in bf16: host pre-casts inputs and decodes outputs
   (tolerance is 2e-2; bf16 end-to-end error ~1e-3).  Halves HBM traffic.
 - Host pre-transposes q/k to [d, n] layout and packs 2 batch-heads per
   128 partitions, so the kernel needs NO on-chip transposes and NO DVE
   PSUM->SBUF staging copies at all.
 - PE does only the score matmuls: lhsT = qt window [64,128] stationary,
   rhs = kt 3-window slice [64,384] moving, f32 PSUM out.
 - ACT does exp batched: one activation instr per 3-window PSUM tile
   (FD=1152) reading strided PSUM slots, writing bf16 stage; amortizes
   the ~300ns per-instruction ScalarE overhead (no accum_out: the
   248 per-window instrs + 279ns/accum-read would make ACT ~180us).
 - Softmax row sums via DVE pairwise tree reduction over the staged bf16
   exps (2x packed mode), ~7 instrs per 31-window group instead of
   per-window tensor_reduce (1x) or ACT accum.
 - Output staged per (bh, half): [128 queries, 31 windows, 384] bf16 and
   DMA'd with fully-contiguous 23.8KB per-partition runs to a
   [bh, i, o, j]-layout DRAM tensor; host untransposes to [bh, o, i, j].

Scheduling constraint (from v1, the hard way): every sync wait of a
Matmult lands on the LDWEIGHTS struct which has a single wait slot, so
each PE instruction may wait on at most ONE semaphore.  All input-DMA
waits are therefore soaked by tiny "absorber" matmuls; real matmuls then
only ever wait on ACT (PSUM slot recycling).
"""

import sys

for _p in ("/opt/trn_rl_repo", "/opt/trn_rl_repo/concourse"):
    if _p not in sys.path:
        sys.path.insert(0, _p)

import numpy as np

B, H, N, D = 4, 8, 8192, 64
BH = B * H                      # 32
NCORES = 8
BHC = BH // NCORES              # 4 batch-heads per core
NPAIR = BHC // 2                # 2 partition-packed bh pairs per core
W = 128                         # window size
NW = N // W                     # 64 windows
EXCLUDED = (2, 34)
REMAINING = [i for i in range(NW) if i not in EXCLUDED]
NOUT = len(REMAINING)           # 62
HALF = NOUT // 2                # 31 output windows per staging group
J = 3 * W                       # 384 keys per query window
SCALE = float(D) ** -0.5        # 0.125

SLOT = 512                      # f32 slots so each matmul write is bank-aligned
# windows per PSUM tile, as (count, pool-tag): the two PSUM tiles (4+3 banks
# + 1 scrap = 8) must strictly alternate so PE never waits on the tile ACT is
# still reading; consecutive halves use complementary patterns so the
# alternation also holds across half boundaries.
BATCHES_A = ((4, "ps4"), (3, "ps3"), (4, "ps4"), (3, "ps3"), (4, "ps4"),
             (3, "ps3"), (4, "ps4"), (3, "ps3"), (3, "ps4"))
BATCHES_B = ((3, "ps3"), (4, "ps4"), (3, "ps3"), (4, "ps4"), (3, "ps3"),
             (4, "ps4"), (3, "ps3"), (4, "ps4"), (3, "ps3"))
# All normalize muls stay on DVE.  Offload attempts that FAILED on HW:
#  - Pool/gpsimd tensor ops share physical SBUF ports with DVE; running both
#    concurrently destroys both (313ns -> 3740ns per DVE mul, wall 3.9x).
#  - ACT SBUF-source ops hit the TRN2 silicon errata (~2.3x slower than
#    spec): measured 1180ns per mul vs 313ns on DVE (wall 174 -> 206us).
MUL_PATTERN = ("v",)

_cached_nc = None


def _build():
    import concourse.bass as bass
    import concourse.mybir as mybir
    import concourse.tile as tile
    from concourse import bacc
    from concourse.tile import add_dep_helper

    fp32 = mybir.dt.float32
    bf16 = mybir.dt.bfloat16
    nc = bacc.Bacc("TRN2", target_bir_lowering=False, debug=False)
    # host-packed: pair p holds bh 2p on partitions 0:64, bh 2p+1 on 64:128,
    # already transposed to [d, n]
    qt = nc.dram_tensor("qt", [NPAIR, 2 * D, N], bf16, kind="ExternalInput").ap()
    kt = nc.dram_tensor("kt", [NPAIR, 2 * D, N], bf16, kind="ExternalInput").ap()
    # i-major output; host untransposes (bh, i, o, j) -> (bh, o, i, j)
    out = nc.dram_tensor("out", [BHC, W, NOUT, J], bf16, kind="ExternalOutput").ap()

    def raw(inst):
        return inst.ins if hasattr(inst, "ins") and not isinstance(inst.ins, list) else inst

    with tile.TileContext(nc) as tc:
        from contextlib import ExitStack

        with ExitStack() as ctx:
            singles = ctx.enter_context(tc.tile_pool(name="singles", bufs=1))
            kin_pool = ctx.enter_context(tc.tile_pool(name="kin", bufs=2))
            qin_pool = ctx.enter_context(tc.tile_pool(name="qin", bufs=2))
            stage_pool = ctx.enter_context(tc.tile_pool(name="stage", bufs=3))
            scr_pool = ctx.enter_context(tc.tile_pool(name="scr", bufs=2))
            sums_pool = ctx.enter_context(tc.tile_pool(name="sums", bufs=2))
            mpsum = ctx.enter_context(tc.tile_pool(name="mpsum", bufs=1, space="PSUM"))
            scrapp = ctx.enter_context(tc.tile_pool(name="scrap", bufs=1, space="PSUM"))

            zeros = singles.tile([128, 128], bf16)
            nc.gpsimd.memset(zeros, 0.0)
            scrap = scrapp.tile([2, 2], fp32, tag="scrap")
            # absorb the gpsimd (zeros) wait into PE's clock once
            ab0 = nc.tensor.matmul(scrap, zeros[:, :2], zeros[:, :2],
                                   start=True, stop=True)

            def absorber(t):
                """1-wait PE matmul absorbing tile t's DMA completion."""
                return nc.tensor.matmul(scrap, t[:, :2], t[:, :2],
                                        start=True, stop=True)

            # chunked input loads: a small head chunk so the first windows'
            # matmuls can start ~3us in, then two big chunks
            CHUNK_EDGES = (0, 768, 4480, N)

            def load_pair(p):
                """Issue the pair-p input DMAs in chunks; k before q per chunk."""
                ktp = kin_pool.tile([2 * D, N], bf16, tag="kin")
                qtp = qin_pool.tile([2 * D, N], bf16, tag="qin")
                for c in range(len(CHUNK_EDGES) - 1):
                    sl = slice(CHUNK_EDGES[c], CHUNK_EDGES[c + 1])
                    nc.gpsimd.dma_start(out=ktp[:, sl], in_=kt[p, :, sl])
                    nc.gpsimd.dma_start(out=qtp[:, sl], in_=qt[p, :, sl])
                return ktp, qtp

            tiles = [load_pair(0)]  # prefetched ahead of compute

            for p in range(NPAIR):
                ktp, qtp = tiles[p]
                if p + 1 < NPAIR:
                    tiles.append(load_pair(p + 1))
                # per-chunk DMA-wait absorbers
                ab_k = [absorber(ktp[:, e:]) for e in CHUNK_EDGES[:-1]]
                ab_q = [absorber(qtp[:, e:]) for e in CHUNK_EDGES[:-1]]

                for sub in range(2):
                    po = D * sub
                    bh = 2 * p + sub
                    for h in range(2):
                        o0 = HALF * h
                        stage = stage_pool.tile([128, HALF, J], bf16, tag="stage")
                        edge_memsets = []
                        b0 = 0
                        half_idx = ((2 * p + sub) * 2 + h)
                        batches = BATCHES_A if half_idx % 2 == 0 else BATCHES_B
                        for bs, ptag in batches:
                            psum = mpsum.tile([128, bs, SLOT], fp32, tag=ptag)
                            for n_ in range(bs):
                                o = o0 + b0 + n_
                                wi = REMAINING[o]
                                q_l = qtp[po:po + D, wi * W:(wi + 1) * W]
                                if wi == 0:
                                    mm = nc.tensor.matmul(
                                        psum[:, n_, W:3 * W], q_l,
                                        ktp[po:po + D, 0:2 * W],
                                        start=True, stop=True,
                                        skip_group_check=True)
                                    zm = nc.tensor.matmul(
                                        psum[:, n_, 0:W], q_l,
                                        zeros[po:po + D, :],
                                        start=True, stop=True,
                                        skip_group_check=True)
                                    add_dep_helper(raw(zm), raw(ab0), False, "zm0")
                                    add_dep_helper(raw(zm), raw(ab_q[0]), False, "zmq")
                                    edge_memsets.append((b0 + n_, 0, W))
                                elif wi == NW - 1:
                                    mm = nc.tensor.matmul(
                                        psum[:, n_, 0:2 * W], q_l,
                                        ktp[po:po + D, (NW - 2) * W:],
                                        start=True, stop=True,
                                        skip_group_check=True)
                                    zm = nc.tensor.matmul(
                                        psum[:, n_, 2 * W:3 * W], q_l,
                                        zeros[po:po + D, :],
                                        start=True, stop=True,
                                        skip_group_check=True)
                                    add_dep_helper(raw(zm), raw(ab0), False, "zm1")
                                    add_dep_helper(raw(zm), raw(ab_q[-1]), False, "zmq")
                                    edge_memsets.append((b0 + n_, 2 * W, 3 * W))
                                else:
                                    mm = nc.tensor.matmul(
                                        psum[:, n_, 0:J], q_l,
                                        ktp[po:po + D, (wi - 1) * W:(wi + 2) * W],
                                        start=True, stop=True)
                                # dep on the DMA chunk(s) this window touches
                                klo = max(wi - 1, 0) * W
                                khi = min(wi + 2, NW) * W
                                for c in range(len(CHUNK_EDGES) - 1):
                                    c0_, c1_ = CHUNK_EDGES[c], CHUNK_EDGES[c + 1]
                                    if klo < c1_ and khi > c0_:
                                        add_dep_helper(raw(mm), raw(ab_k[c]),
                                                       False, "mmk")
                                    if wi * W < c1_ and (wi + 1) * W > c0_:
                                        add_dep_helper(raw(mm), raw(ab_q[c]),
                                                       False, "mmq")
                            # batched exp: strided read of the bs psum slots
                            nc.scalar.activation(
                                stage[:, b0:b0 + bs, :],
                                psum[:, :bs, 0:J],
                                mybir.ActivationFunctionType.Exp,
                                scale=SCALE,
                            )
                            b0 += bs

                        # zero the padded edge regions (before row sums)
                        for (oo, j0, j1) in edge_memsets:
                            nc.vector.memset(stage[:, oo, j0:j1], 0.0)

                        # pairwise-tree row sums over j: 384 -> 3 -> 1
                        scr = scr_pool.tile([128, HALF, J // 2], bf16, tag="scr")
                        nc.vector.tensor_add(
                            scr, stage[:, :, 0:192], stage[:, :, 192:384])
                        wdt = 96
                        while wdt >= 3:
                            nc.vector.tensor_add(
                                scr[:, :, 0:wdt], scr[:, :, 0:wdt],
                                scr[:, :, wdt:2 * wdt])
                            wdt //= 2
                        sums = sums_pool.tile([128, HALF], fp32, tag="sums")
                        nc.vector.tensor_add(
                            sums, scr[:, :, 0], scr[:, :, 1])
                        nc.vector.tensor_add(sums, sums, scr[:, :, 2])
                        recip = sums_pool.tile([128, HALF], fp32, tag="recip")
                        nc.vector.reciprocal(recip, sums)
                        for oo in range(HALF):
                            nc.vector.tensor_scalar_mul(
                                stage[:, oo, :], stage[:, oo, :],
                                recip[:, oo:oo + 1])
                            if oo == 15:
                                # first 16 windows ship while the rest
                                # normalize: shortens the per-half DMA tail
                                nc.sync.dma_start(
                                    out=out[bh, :, o0:o0 + 16, :],
                                    in_=stage[:, 0:16, :])
                        nc.sync.dma_start(
                            out=out[bh, :, o0 + 16:o0 + HALF, :],
                            in_=stage[:, 16:HALF, :])
    nc.compile()
    return nc


def _pack_inputs(x):
    """(BH, N, D) f32 -> per-core [NPAIR, 128, N] bf16, d-major."""
    from ml_dtypes import bfloat16

    x = np.ascontiguousarray(np.asarray(x), dtype=np.float32).reshape(BH, N, D)
    x = x.astype(bfloat16)
    per_core = []
    for c in range(NCORES):
        a = x[c * BHC:(c + 1) * BHC]              # [4, N, D]
        a = a.transpose(0, 2, 1)                  # [4, D, N]
        a = np.ascontiguousarray(a).reshape(NPAIR, 2 * D, N)
        per_core.append(a)
    return per_core


def _run(q, k, trace=False):
    from concourse.bass_utils import run_bass_kernel_spmd

    global _cached_nc
    if _cached_nc is None:
        _cached_nc = _build()
    nc = _cached_nc

    qs = _pack_inputs(q)
    ks = _pack_inputs(k)
    in_maps = [{"qt": qs[c], "kt": ks[c]} for c in range(NCORES)]
    res = run_bass_kernel_spmd(nc, in_maps, core_ids=list(range(NCORES)), trace=trace)
    outs = []
    for c in range(NCORES):
        o = np.asarray(res.results[c]["out"])     # [BHC, W, NOUT, J] bf16
        outs.append(o.astype(np.float32).transpose(0, 2, 1, 3))
    full = np.concatenate(outs, axis=0)           # [BH, NOUT, W, J]
    return np.ascontiguousarray(full), res


def kernel(q, k):
    out, _ = _run(q, k, trace=False)
    return out
